# revision 8
# baseline (speedup 1.0000x reference)
"""Causal multi-head attention on 8 Trainium2 NeuronCores.

Sharding: 8 cores = 4 batches x 2 head-halves. Each core handles one batch
and 8 of the 16 heads (feature range hf*512 .. hf*512+512 of the QKV
projections), computes a partial output projection [2048, 1024], and the
host sums the two half-partials per batch and adds the bias.

Per-core kernel (v2):
  - single sweep over x.T chunks computes qT/kT (bf16, [128, 2048] per
    head-pair, heads stacked 64+64 on partitions) and v (bf16,
    [tok, 17, head, 64]: 16 token-blocks + a shared ones-block)
  - attention qt-outer / head-pair-inner; per 128-key block j the two heads
    of the pair run as a tile_position ping-pong (rows 0:63 / 64:127), both
    into one PSUM tile [128, 2, 512]; exp on ACT (scale 1/8 folded) narrowed
    to the causal range; 0/1 mask mult on DVE for diagonal blocks
  - PV per block: lhsT = [v | ones] (M=128: PSUM rows 0:63 ctx, rows
    64:127 the softmax denominator replicated across 64 partitions),
    deferred 8 blocks behind exp; per (hp,h): reciprocal straight from the
    PSUM denominator rows, ctx copy, gpsimd normalize into the fp32r ctile
  - proj/out-proj work is deadline-paced in ~2-matmul thunks into the
    attention slots so the PE stream never starves; DMA queues are split
    (x chunks: sync, weights: scalar, outputs: vector)
"""

import numpy as np

B, S, D = 4, 2048, 1024

_CACHE = {}
PEND_DEPTH = 8


def _build(R=1, mode="full"):
    import concourse.bacc as bacc
    import concourse.tile as tile
    import concourse.mybir as mybir
    from concourse.bass import ts, ds

    fr = mybir.dt.float32r
    f32 = mybir.dt.float32
    bf = mybir.dt.bfloat16
    Exp = mybir.ActivationFunctionType.Exp
    Alu = mybir.AluOpType

    nc = bacc.Bacc("TRN2", target_bir_lowering=False)
    xT_d = nc.dram_tensor("xT", [128, 8, S], bf, kind="ExternalInput")
    wq_d = nc.dram_tensor("wq", [128, 4, 8, 128], bf, kind="ExternalInput")
    wk_d = nc.dram_tensor("wk", [128, 4, 8, 128], bf, kind="ExternalInput")
    wv_d = nc.dram_tensor("wv", [128, 8, 512], bf, kind="ExternalInput")
    wp_d = nc.dram_tensor("wp", [128, 4, 1024], fr, kind="ExternalInput")
    mk01_d = nc.dram_tensor("mask01", [128, 128], bf, kind="ExternalInput")
    out_d = nc.dram_tensor("out", [16, 128, 1024], f32, kind="ExternalOutput")

    with tile.TileContext(nc) as tc:
        with (
            tc.tile_pool(name="const", bufs=1) as cpool,
            tc.tile_pool(name="chunks", bufs=2) as chp,
            tc.tile_pool(name="xp", bufs=12) as xpp,
            tc.tile_pool(name="rrp", bufs=2) as rrp,
            tc.tile_pool(name="ctxt", bufs=3) as ctxtp,
            tc.tile_pool(name="ost", bufs=2) as ostp,
            tc.tile_pool(name="sc", bufs=2, space="PSUM") as scp,
            tc.tile_pool(name="ctx", bufs=2, space="PSUM") as ctxp,
            tc.tile_pool(name="pj", bufs=2, space="PSUM") as pjp,
        ):
            m01_sb = cpool.tile([128, 128], bf, tag="m01")
            nc.scalar.dma_start(m01_sb[:], mk01_d[:])
            ones32 = cpool.tile([128, 128], f32, tag="ones32")
            nc.vector.memset(ones32[:], 1.0)
            # v: [tok-part, tok-block, head, 128]: cols 0:63 = v, 64:127 = 1.0
            # (ones give PV M=128: PSUM rows 64:127 = replicated denominator)
            v_sb = cpool.tile([128, 16, 8, 128], bf, tag="v")
            nc.gpsimd.memset(v_sb[:, :, :, 64:128], 1.0)
            # persistent qT/kT for all 4 head-pairs (heads 64+64 on partitions)
            qTs, kTs = [], []
            for hp in range(4):
                qTs.append(cpool.tile([128, S], bf, tag=f"qT{hp}", name=f"qT{hp}"))
                kTs.append(cpool.tile([128, S], bf, tag=f"kT{hp}", name=f"kT{hp}"))
            wv_sb = cpool.tile([128, 8, 512], bf, tag="wv")
            wp_sb = cpool.tile([128, 4, 1024], fr, tag="wp")
            wqt_sb, wkt_sb = [], []
            for hp in range(4):
                wqt_sb.append(
                    cpool.tile([128, 8, 128], bf, tag=f"wq{hp}", name=f"wq{hp}")
                )
                wkt_sb.append(
                    cpool.tile([128, 8, 128], bf, tag=f"wk{hp}", name=f"wk{hp}")
                )

            def proj_setup():
                # weights stream on the ACT HWDGE queue; x chunks go on the
                # sync queue so the first matmul's two inputs arrive in
                # parallel
                for hp in range(4):
                    nc.scalar.dma_start(wqt_sb[hp][:], wq_d[:, hp, :, :])
                    nc.scalar.dma_start(wkt_sb[hp][:], wk_d[:, hp, :, :])
                nc.scalar.dma_start(wv_sb[:], wv_d[:])
                nc.scalar.dma_start(wp_sb[:], wp_d[:])

            def proj_cb_thunks(cb):
                """Fill thunks for x-chunk cb: one DMA thunk + 2-matmul
                compute thunks (qk groups split in 4, v groups in 4)."""
                box = {}

                def dma_chunk():
                    ch = chp.tile([128, 8, 512], bf, tag="ch")
                    for kc in range(8):
                        nc.sync.dma_start(ch[:, kc, :], xT_d[:, kc, ts(cb, 512)])
                    box["ch"] = ch

                thunks = [dma_chunk]

                def qk_part(wt, dst, part):
                    ch = box["ch"]
                    if part == 0:
                        box["pq"] = pjp.tile([128, 512], f32, tag="pj", name="pq")
                    pq = box["pq"]
                    for kc in (2 * part, 2 * part + 1):
                        nc.tensor.matmul(
                            pq[:],
                            wt[:, kc, :],
                            ch[:, kc, :],
                            start=(kc == 0),
                            stop=(kc == 7),
                        )
                    if part == 3:
                        nc.vector.tensor_copy(dst[:, ts(cb, 512)], pq[:])

                def v_part(sb4, part):
                    ch = box["ch"]
                    tb = cb * 4 + sb4
                    if part == 0:
                        box["pv"] = pjp.tile([128, 8, 64], f32, tag="pj", name="pv")
                    pv = box["pv"]
                    for kc in (2 * part, 2 * part + 1):
                        nc.tensor.matmul(
                            pv[:],
                            ch[:, kc, ts(sb4, 128)],
                            wv_sb[:, kc, :],
                            start=(kc == 0),
                            stop=(kc == 7),
                        )
                    if part == 3:
                        nc.vector.tensor_copy(v_sb[:, tb, :, 0:64], pv[:])

                import functools

                for hp in range(4):
                    for wt, dst in (
                        (wqt_sb[hp], qTs[hp]),
                        (wkt_sb[hp], kTs[hp]),
                    ):
                        for part in range(4):
                            thunks.append(functools.partial(qk_part, wt, dst, part))
                for sb4 in range(4):
                    for part in range(4):
                        thunks.append(functools.partial(v_part, sb4, part))
                return thunks

            def proj_cb(cb):
                for t in proj_cb_thunks(cb):
                    t()

            class Pacer:
                """Consume fill thunks evenly across the remaining slots."""

                def __init__(self, thunks, slots):
                    from collections import deque

                    self.q = deque(thunks)
                    self.slots = slots

                def slot(self):
                    if self.slots > 0:
                        take = -(-len(self.q) // self.slots)  # ceil
                        self.slots -= 1
                        for _ in range(take):
                            if self.q:
                                self.q.popleft()()

                def drain(self):
                    while self.q:
                        self.q.popleft()()
                    self.slots = 0

            def attn_qt(hp, qt, pacer, ctile):
                """Attention for one (head-pair, query-tile of 512)."""
                qT, kT = qTs[hp], kTs[hp]
                ctx = {
                    h: ctxp.tile([128, 512], f32, tag="ctx", name=f"ctx{h}")
                    for h in (0, 1)
                }
                n_kb = 4 * qt + 4
                pend = []

                def emit_pv(item):
                    j, xp_, qo, last = item
                    for h in (0, 1):
                        nc.tensor.matmul(
                            ctx[h][:, qo:512],
                            v_sb[:, j, 2 * hp + h, :],
                            xp_[:, h, qo:512],
                            start=(j == 0),
                            stop=last,
                            skip_group_check=True,
                        )

                for j in range(n_kb):
                    m = j - 4 * qt
                    qoff = 128 * m if m > 0 else 0
                    sc = scp.tile([128, 2, 512], f32, tag="sc")
                    for h in (0, 1):
                        nc.tensor.matmul(
                            sc[:, h, qoff:512],
                            kT[ds(64 * h, 64), ts(j, 128)],
                            qT[ds(64 * h, 64), ds(qt * 512 + qoff, 512 - qoff)],
                            start=True,
                            stop=True,
                            tile_position=(64 * h, 0),
                        )
                    xp = xpp.tile([128, 2, 512], bf, tag="xp")
                    nc.scalar.activation(
                        xp[:, :, qoff:512], sc[:, :, qoff:512], Exp, scale=0.125
                    )
                    if m >= 0:
                        for h in (0, 1):
                            nc.vector.tensor_tensor(
                                out=xp[:, h, ds(qoff, 128)],
                                in0=xp[:, h, ds(qoff, 128)],
                                in1=m01_sb[:],
                                op=Alu.mult,
                            )
                    pend.append((j, xp, qoff, j == n_kb - 1))
                    while len(pend) > PEND_DEPTH:
                        emit_pv(pend.pop(0))
                    pacer.slot()
                while pend:
                    emit_pv(pend.pop(0))
                # normalize into ctile: reciprocal of the replicated denominator
                # rows straight from PSUM, ctx copy, gpsimd multiply
                for h in (0, 1):
                    rr = rrp.tile([128, 512], fr, tag="rr", name=f"rr{h}")
                    with nc.allow_low_precision(reason="fp32r recip for mult"):
                        nc.vector.reciprocal(rr[ds(64 * h, 64), :], ctx[h][ds(64, 64), :])
                    dst = ctile[ds(64 * h, 64), hp, :]
                    nc.vector.tensor_copy(dst, ctx[h][0:64, :])
                    nc.gpsimd.tensor_tensor(
                        out=dst, in0=dst, in1=rr[ds(64 * h, 64), :], op=Alu.mult
                    )

            def out_proj_thunks(qt, ctile):
                import functools

                box = {}

                def op_part(tt, ncv, part):
                    if part == 0:
                        box[(tt, ncv)] = pjp.tile([128, 512], f32, tag="pj", name="po")
                    po = box[(tt, ncv)]
                    for fc in (2 * part, 2 * part + 1):
                        nc.tensor.matmul(
                            po[:],
                            ctile[:, fc, ts(tt % 4, 128)],
                            wp_sb[:, fc, ds(ncv * 512, 512)],
                            start=(fc == 0),
                            stop=(fc == 3),
                        )
                    if part == 1:
                        ot = ostp.tile([128, 512], f32, tag="ost")
                        nc.vector.tensor_copy(ot[:], po[:])
                        nc.scalar.dma_start(out_d[tt, :, ds(ncv * 512, 512)], ot[:])

                return [
                    functools.partial(op_part, qt * 4 + tt4, ncv, part)
                    for tt4 in range(4)
                    for ncv in range(2)
                    for part in range(2)
                ]

            def out_proj_qt(qt, ctile):
                for t in out_proj_thunks(qt, ctile):
                    t()

            def body():
                proj_setup()
                proj_cb(0)
                cts = []
                for qt in range(4):
                    ctile = ctxtp.tile([128, 4, 512], fr, tag="ctxt", name=f"ct{qt}")
                    cts.append(ctile)
                    thunks = []
                    if qt < 3:
                        thunks.extend(proj_cb_thunks(qt + 1))
                    if qt == 2:
                        thunks.extend(out_proj_thunks(0, cts[0]))
                    elif qt == 3:
                        thunks.extend(out_proj_thunks(1, cts[1]))
                        thunks.extend(out_proj_thunks(2, cts[2]))
                    pacer = Pacer(thunks, 4 * (4 * qt + 4))
                    for hp in range(4):
                        attn_qt(hp, qt, pacer, ctile)
                    pacer.drain()
                out_proj_qt(3, cts[3])

            if R > 1:
                with tc.For_i(0, R):
                    body()
            else:
                body()

    nc.compile()
    return nc


def _get_program(R=1, mode="full"):
    key = (R, mode)
    if key not in _CACHE:
        _CACHE[key] = _build(R, mode)
    return _CACHE[key]


def _shard_inputs(x, Wq, Wk, Wv, Wp):
    import ml_dtypes

    bf = ml_dtypes.bfloat16
    x = np.ascontiguousarray(x, dtype=np.float32)
    mask01 = np.where(
        np.arange(128)[:, None] > np.arange(128)[None, :], 0.0, 1.0
    ).astype(bf)
    in_maps = []
    for c in range(8):
        b, hf = c // 2, c % 2
        hs = slice(hf * 512, hf * 512 + 512)
        xT = np.ascontiguousarray(
            x[b].T.reshape(8, 128, S).transpose(1, 0, 2)
        ).astype(bf)
        wq = np.ascontiguousarray(
            Wq[hs].T.reshape(8, 128, 4, 128).transpose(1, 2, 0, 3)
        ).astype(bf)
        wk = np.ascontiguousarray(
            Wk[hs].T.reshape(8, 128, 4, 128).transpose(1, 2, 0, 3)
        ).astype(bf)
        wv = np.ascontiguousarray(
            Wv[hs].T.reshape(8, 128, 512).transpose(1, 0, 2)
        ).astype(bf)
        wp = np.ascontiguousarray(Wp.T[hs].reshape(4, 128, D).transpose(1, 0, 2))
        in_maps.append(
            {"xT": xT, "wq": wq, "wk": wk, "wv": wv, "wp": wp, "mask01": mask01}
        )
    return in_maps


def kernel(x, Wq, Wk, Wv, Wp, bp, _R=1, _return_res=False):
    from concourse.bass_utils import run_bass_kernel_spmd

    nc = _get_program(_R)
    in_maps = _shard_inputs(x, Wq, Wk, Wv, Wp)
    res = run_bass_kernel_spmd(nc, in_maps, list(range(8)))
    out = np.empty((B, S, D), dtype=np.float32)
    for b in range(B):
        p0 = res.results[2 * b]["out"].reshape(S, D)
        p1 = res.results[2 * b + 1]["out"].reshape(S, D)
        out[b] = p0 + p1 + bp.astype(np.float32)
    if _return_res:
        return out, res
    return out
